# revision 8
# baseline (speedup 1.0000x reference)
"""TRN2 Bass kernel for nn_AddSparseAndLowRankCorrection.

Math:  out = x @ W_eff^T  with  W_eff = W_inner + A @ Bmat + S,
where S is the coalesced (duplicate-summing) dense form of the COO input
(sp_indices, sp_values).

Sharding (8 NeuronCores): tensor-parallel over the output dimension —
core c owns output columns [512c, 512c+512). x (transposed, bf16) is
replicated; W_inner / A are sharded by output rows; Bmat replicated; the
COO entries are sharded by output row and re-laid-out per core (pure
index/layout work on the host — all value arithmetic happens on device).

Per-core device graph (PSUM accumulation in fp32, operands bf16):

Phase A - build W_eff^T as 32 chunk tiles w_chunks[k] ([128, 512] bf16):
  - dup-fold: duplicate COO values are expanded level-major in dupx
    [128, L*CW] (L=4 levels); 3 DVE adds fold them into sdat's
    merged-dup columns, so every duplicate slot carries its on-device
    sum before the scatter.
  - 16 gpsimd local_scatter calls densify the per-(partition, chunk)
    sparse slots into sden staging tiles ([128, 1024] bf16).
  - per chunk k: ps = (A@Bmat)^T chunk via PE (lhsT=Bmat[:,chunk],
    rhs=A^T); copy psum->SBUF (alternating ACT/DVE to double the
    pipeline rate); += W_inner^T chunk (DMA, bf16); w_chunks[k] =
    that + sden slice.

Main loop - weight-stationary GEMM, transposed output:
  outT[o, i] = sum_d W_eff^T[d, o] * xT[d, i].
  For each i-block of 2048 (4 blocks), x chunk tiles [128, 2048] are
  SBUF-resident; for each 128-wide output sub-block m, the stationary
  operand w_chunks[k][:, m*128:(m+1)*128] is loaded once and FOUR
  512-column moving streams run against it (4 PSUM banks). This
  amortizes LDWEIGHTS 4x vs the x-stationary form (measured 415 us vs
  508 us pure-PE for the same 2048 matmuls). Output drains as bf16 to
  a transposed DRAM tensor; the host transposes back and upcasts.
"""
import sys

sys.path.insert(0, "/opt/trn_rl_repo")

import numpy as np
import ml_dtypes

P = 128
D = 4096          # d_in (contraction dim)
D_OUT = 4096
NI = 8192         # 4*2048 flattened x rows
O = 512           # output columns per core
KC = D // P       # 32 d-chunks
N_CORES = 8

NS0, C, L = 56, 8, 4     # per-row unique slots / dup slots / max multiplicity
SG = 1                   # d-chunks per local_scatter call
NS = NS0 + C             # 64
CW = KC * C              # dup columns per fold level

IB = 2048                # i-block: moving-stream span per stationary load
NB = NI // IB            # 4 i-blocks
NSUB = IB // 512         # 4 moving streams (PSUM banks) per stationary load

_COMPILED = {}


def _build(n_loop: int = 1):
    import contextlib

    import concourse.bacc as bacc
    import concourse.mybir as mybir
    import concourse.tile as tile

    F32 = mybir.dt.float32
    BF16 = mybir.dt.bfloat16
    I16 = mybir.dt.int16

    nc = bacc.Bacc("TRN2", target_bir_lowering=False, debug=False)
    xT = nc.declare_dram_parameter("xT", [D, NI], BF16, isOutput=False)
    wbT = nc.declare_dram_parameter("wbT", [D, O], BF16, isOutput=False)
    abT = nc.declare_dram_parameter("abT", [64, O], BF16, isOutput=False)
    bm = nc.declare_dram_parameter("bm", [64, D], BF16, isOutput=False)
    sdat = nc.declare_dram_parameter("sdat", [P, KC * NS], BF16, isOutput=False)
    sidx = nc.declare_dram_parameter("sidx", [P, KC * NS], I16, isOutput=False)
    dupx = nc.declare_dram_parameter("dupx", [P, L * CW], BF16, isOutput=False)
    outT = nc.declare_dram_parameter("outT", [O, NI], BF16, isOutput=True)

    with tile.TileContext(nc) as tc:
        # n_loop > 1 wraps the body in an in-NEFF hardware loop for
        # loop-differencing timing (see test.py).
        loop_cm = tc.For_i(0, n_loop) if n_loop > 1 else contextlib.nullcontext()
        with (
            loop_cm,
            tc.tile_pool(name="wconst", bufs=1) as wconst,
            tc.tile_pool(name="opool", bufs=6) as opool,
            tc.tile_pool(name="ppool", bufs=8, space="PSUM") as ppool,
        ):
            # W_eff^T chunks as separate tiles: precise per-chunk deps so
            # the main loop starts as soon as chunk 0 is ready.
            w_chunks = []
            for k in range(KC):
                wck = wconst.tile([P, O], BF16, tag=f"wc{k}")
                w_chunks.append(wck)

            with (
                tc.tile_pool(name="scpool", bufs=1) as scpool,
                tc.tile_pool(name="sdpool", bufs=4) as sdpool,
                tc.tile_pool(name="wpool", bufs=3) as wpool,
                tc.tile_pool(name="tpool", bufs=3) as tpool,
            ):
                dupx_s = scpool.tile([P, L * CW], BF16, tag="dupx")
                nc.sync.dma_start(out=dupx_s[:], in_=dupx[:])
                sdat_s = scpool.tile([P, KC * NS], BF16, tag="sdat")
                nc.scalar.dma_start(out=sdat_s[:], in_=sdat[:])
                sidx_s = scpool.tile([P, KC * NS], I16, tag="sidx")
                nc.scalar.dma_start(out=sidx_s[:], in_=sidx[:])
                abT_s = scpool.tile([64, O], BF16, tag="abT")
                nc.scalar.dma_start(out=abT_s[:], in_=abT[:])
                bm_s = scpool.tile([64, D], BF16, tag="bm")
                nc.sync.dma_start(out=bm_s[:], in_=bm[:])

                # dup-fold: 3 adds; the last writes straight into sdat's
                # merged-dup columns (strided 3D view).
                acc = scpool.tile([P, CW], BF16, tag="acc")
                nc.vector.tensor_add(acc[:], dupx_s[:, 0:CW],
                                     dupx_s[:, CW:2 * CW])
                acc2 = scpool.tile([P, CW], BF16, tag="acc2")
                nc.vector.tensor_add(acc2[:], acc[:],
                                     dupx_s[:, 2 * CW:3 * CW])
                sd3 = sdat_s[:].rearrange("p (k n) -> p k n",
                                          n=NS)[:, :, NS0:NS]
                ac3 = acc2[:].rearrange("p (k c) -> p k c", c=C)
                lv3 = dupx_s[:, 3 * CW:4 * CW].rearrange(
                    "p (k c) -> p k c", c=C)
                nc.vector.tensor_add(sd3, ac3, lv3)

                sdens = {}
                for g in range(KC // SG):
                    sden = sdpool.tile([P, SG * O], BF16, tag="sden")
                    nc.gpsimd.local_scatter(
                        out_ap=sden[:],
                        data_ap=sdat_s[:, g * SG * NS:(g + 1) * SG * NS],
                        idxs_ap=sidx_s[:, g * SG * NS:(g + 1) * SG * NS],
                        channels=P, num_elems=SG * O, num_idxs=SG * NS)
                    sdens[g] = sden

                for k in range(KC):
                    sl = slice(k * P, (k + 1) * P)
                    wt = wpool.tile([P, O], BF16, tag="wt")
                    nc.scalar.dma_start(out=wt[:], in_=wbT[sl, :])
                    ps = ppool.tile([P, O], F32, tag="acc")
                    nc.tensor.matmul(ps[:], lhsT=bm_s[:, sl], rhs=abT_s[:],
                                     start=True, stop=True)
                    pst = tpool.tile([P, O], BF16, tag="pst")
                    if k % 2 == 0:
                        nc.scalar.copy(out=pst[:], in_=ps[:])
                    else:
                        nc.vector.tensor_copy(pst[:], ps[:])
                    nc.vector.tensor_add(pst[:], pst[:], wt[:])
                    g, j = k // SG, k % SG
                    nc.vector.tensor_add(w_chunks[k][:], pst[:],
                                         sdens[g][:, j * O:(j + 1) * O])

            # main GEMM: weight-stationary, 4 moving streams per load
            with tc.tile_pool(name="xpool", bufs=36) as xpool:
                for ib in range(NB):
                    xts = []
                    for k in range(KC):
                        xt = xpool.tile([P, IB], BF16, tag="xt")
                        nc.sync.dma_start(
                            out=xt[:],
                            in_=xT[k * P:(k + 1) * P, ib * IB:(ib + 1) * IB])
                        xts.append(xt)
                    for m in range(4):
                        psl = []
                        for _n in range(NSUB):
                            pt = ppool.tile([P, 512], F32, tag="acc")
                            psl.append(pt)
                        for k in range(KC):
                            wsl = w_chunks[k][:, m * P:(m + 1) * P]
                            for n in range(NSUB):
                                nc.tensor.matmul(
                                    psl[n][:], lhsT=wsl,
                                    rhs=xts[k][:, n * 512:(n + 1) * 512],
                                    start=(k == 0), stop=(k == KC - 1))
                        for n in range(NSUB):
                            ot = opool.tile([P, 512], BF16, tag="ot")
                            nc.vector.tensor_copy(ot[:], psl[n][:])
                            c0 = ib * IB + n * 512
                            nc.scalar.dma_start(
                                out=outT[m * P:(m + 1) * P, c0:c0 + 512],
                                in_=ot[:])

    nc.compile()
    return nc


def _cumcount(keys):
    order = np.argsort(keys, kind="stable")
    ks = keys[order]
    _, st, ct = np.unique(ks, return_index=True, return_counts=True)
    oc = np.arange(len(ks)) - np.repeat(st, ct)
    res = np.empty(len(keys), dtype=np.int64)
    res[order] = oc
    return res


def _host_prep(x, W_inner, A, Bmat, sp_values, sp_indices):
    """Shard + layout-prep full inputs -> per-core in_maps.

    Pure layout/index manipulation; no value arithmetic beyond dtype cast.
    """
    x2 = np.asarray(x, dtype=np.float32).reshape(NI, D)
    xT = np.ascontiguousarray(x2.T).astype(ml_dtypes.bfloat16)
    W = np.asarray(W_inner, dtype=np.float32)
    A = np.asarray(A, dtype=np.float32)
    B = np.asarray(Bmat, dtype=np.float32)
    vals = np.asarray(sp_values, dtype=np.float32)
    spi = np.asarray(sp_indices)          # to host before slicing: indexing a
    rows = spi[0].astype(np.int64)        # jax array would trigger a neuron
    cols = spi[1].astype(np.int64)        # jit compile of dynamic_slice
    bmx = B.astype(ml_dtypes.bfloat16)

    in_maps = []
    for c in range(N_CORES):
        o0 = c * O
        wbT = np.ascontiguousarray(W[o0:o0 + O, :].T).astype(ml_dtypes.bfloat16)
        abT = np.ascontiguousarray(A[o0:o0 + O, :].T).astype(ml_dtypes.bfloat16)

        msk = (rows >= o0) & (rows < o0 + O)
        d = cols[msk]
        o = rows[msk] - o0
        v = vals[msk]
        slot = d * O + o
        order = np.argsort(slot, kind="stable")
        ds, os_, vs, slots = d[order], o[order], v[order], slot[order]
        uniq, starts, counts = np.unique(slots, return_index=True,
                                         return_counts=True)
        occ = np.arange(len(slots)) - np.repeat(starts, counts)
        mult = np.repeat(counts, counts)
        assert counts.max() <= L, (
            f"core {c}: COO multiplicity {counts.max()} exceeds L={L}")

        sdat = np.zeros((P, KC * NS), dtype=np.float32)
        sidx = np.full((P, KC * NS), -1, dtype=np.int16)
        dupx = np.zeros((P, L * CW), dtype=np.float32)

        dk = (ds // P).astype(np.int64)
        dp = (ds % P).astype(np.int64)
        bucket = dp * KC + dk

        uq = mult == 1
        posu = _cumcount(np.where(uq, bucket, -1))
        if uq.any():
            assert posu[uq].max() < NS0, (
                f"core {c}: {posu[uq].max() + 1} unique entries/row > {NS0}")
        pu, ku, qu = dp[uq], dk[uq], posu[uq]
        sdat[pu, ku * NS + qu] = vs[uq]
        sidx[pu, ku * NS + qu] = os_[uq] + O * (ku % SG)

        rep = (occ == 0) & (mult > 1)
        posd = _cumcount(np.where(rep, bucket, -1))
        if rep.any():
            assert posd[rep].max() < C, (
                f"core {c}: {posd[rep].max() + 1} dup slots/row > {C}")
        dslot_col = np.full(len(slots), -1, dtype=np.int64)
        dslot_col[rep] = posd[rep]
        grp = np.repeat(np.arange(len(uniq)), counts)
        rep_col = np.full(len(uniq), -1, dtype=np.int64)
        rep_col[counts > 1] = dslot_col[starts[counts > 1]]
        ecol = rep_col[grp]
        dup = mult > 1
        pd_, kd_, cd_, ld_ = dp[dup], dk[dup], ecol[dup], occ[dup]
        dupx[pd_, ld_ * CW + kd_ * C + cd_] = vs[dup]
        pr, kr, cr = dp[rep], dk[rep], posd[rep]
        sidx[pr, kr * NS + NS0 + cr] = os_[rep] + O * (kr % SG)

        in_maps.append({
            "xT": xT, "wbT": wbT, "abT": abT, "bm": bmx,
            "sdat": sdat.astype(ml_dtypes.bfloat16),
            "sidx": sidx,
            "dupx": dupx.astype(ml_dtypes.bfloat16),
        })
    return in_maps


def kernel(x, W_inner, A, Bmat, sp_values, sp_indices):
    from concourse.bass_utils import run_bass_kernel_spmd

    in_maps = _host_prep(x, W_inner, A, Bmat, sp_values, sp_indices)
    if "nc" not in _COMPILED:
        _COMPILED["nc"] = _build()
    res = run_bass_kernel_spmd(_COMPILED["nc"], in_maps,
                               core_ids=list(range(N_CORES)))
    full = np.empty((NI, D_OUT), dtype=np.float32)
    for c in range(N_CORES):
        full[:, c * O:(c + 1) * O] = res.results[c]["outT"].T.astype(np.float32)
    return full.reshape(np.asarray(x).shape[:-1] + (D_OUT,))


# revision 9
# speedup vs baseline: 1.0244x; 1.0244x over previous
"""TRN2 Bass kernel for nn_AddSparseAndLowRankCorrection.

Math:  out = x @ W_eff^T  with  W_eff = W_inner + A @ Bmat + S,
where S is the coalesced (duplicate-summing) dense form of the COO input
(sp_indices, sp_values).

Sharding (8 NeuronCores): tensor-parallel over the output dimension —
core c owns output columns [512c, 512c+512). x (transposed, bf16) is
replicated; W_inner / A are sharded by output rows; Bmat replicated; the
COO entries are sharded by output row and re-laid-out per core (pure
index/layout work on the host — all value arithmetic happens on device).

Per-core device graph (PSUM accumulation in fp32, operands bf16):

Phase A — build W_eff^T as 32 chunk tiles w_chunks[k] ([128, 512] bf16),
fully co-resident and overlapped with the start of the main GEMM:
  - dup-fold: duplicate COO values are expanded level-major in dupx
    [128, L*CW] (L=4 levels); 3 DVE adds fold them into sdat's
    merged-dup columns so every duplicate slot carries its on-device sum
    before the scatter.
  - 32 gpsimd local_scatter calls densify the per-(partition, chunk)
    sparse slots into sden staging tiles ([128, 512] bf16; one call per
    chunk measured 2.2x faster than two-chunk calls).
  - per chunk k: W_inner^T chunk DMAs straight into w_chunks[k];
    ps = (A@Bmat)^T chunk via PE (lhsT=Bmat[:,chunk], rhs=A^T) is
    copied out of PSUM (ACT/DVE alternating) and added in place, then
    the sden slice is added in place.

Main GEMM — weight-stationary, transposed output:
  outT[o, i] = sum_d W_eff^T[d, o] * xT[d, i].
  The stationary operand w_chunks[k][:, m*128:(m+1)*128] is loaded once
  per (k, m) and multiple 512-column moving streams of x run against it
  (4 streams for the 2048-row middle i-blocks = measured 415 us pure-PE
  for the 2048 matmuls vs 508 us for the x-stationary form).
  i-block layout [1024 | 2048 | 2048 | 2048 | 1024]:
  - head (1024): m0+m1 k-interleaved (2 streams each, 4 PSUM banks,
    phase A's ps accumulator gets the other 2) with phase-A chunk
    production woven into the same k-loop, so PE alternates ps matmuls
    with main matmuls and the cold-start x-DMA burst is halved.
  - middles (2048): m0+m1 k-interleaved (8 banks, borrowing phase A's 2
    after it finishes), then m2, m3 sequential; x tiles for the next
    block refill during m2/m3 (their slots free as m3 consumes).
  - tail (1024): m01 / m23 paired passes.
  x loads split across the sync/scalar HWDGE rings (measured +27 us for
  single-ring); outT drains as bf16 on the scalar ring (SWDGE stores
  measured far worse). Host transposes outT back and upcasts.
"""
import sys

sys.path.insert(0, "/opt/trn_rl_repo")

import numpy as np
import ml_dtypes

P = 128
D = 4096          # d_in (contraction dim)
D_OUT = 4096
NI = 8192         # 4*2048 flattened x rows
O = 512           # output columns per core
KC = D // P       # 32 d-chunks
N_CORES = 8

NS0, C, L = 56, 8, 4     # per-row unique slots / dup slots / max multiplicity
SG = 1                   # d-chunks per local_scatter call
NS = NS0 + C             # 64
CW = KC * C              # dup columns per fold level

IBH = 1024               # head/tail i-block
IBM = 2048               # middle i-block
LEAD = 6                 # phase-A chunk lead over the head-ib m01 loop

_COMPILED = {}


def _build(n_loop: int = 1, xbufs: int = 32):
    import contextlib

    import concourse.bacc as bacc
    import concourse.mybir as mybir
    import concourse.tile as tile

    F32 = mybir.dt.float32
    BF16 = mybir.dt.bfloat16
    I16 = mybir.dt.int16

    nc = bacc.Bacc("TRN2", target_bir_lowering=False, debug=False)
    xT = nc.declare_dram_parameter("xT", [D, NI], BF16, isOutput=False)
    wbT = nc.declare_dram_parameter("wbT", [D, O], BF16, isOutput=False)
    abT = nc.declare_dram_parameter("abT", [64, O], BF16, isOutput=False)
    bm = nc.declare_dram_parameter("bm", [64, D], BF16, isOutput=False)
    sdat = nc.declare_dram_parameter("sdat", [P, KC * NS], BF16, isOutput=False)
    sidx = nc.declare_dram_parameter("sidx", [P, KC * NS], I16, isOutput=False)
    dupx = nc.declare_dram_parameter("dupx", [P, L * CW], BF16, isOutput=False)
    outT = nc.declare_dram_parameter("outT", [O, NI], BF16, isOutput=True)

    with tile.TileContext(nc) as tc:
        # n_loop > 1 wraps the body in an in-NEFF hardware loop for
        # loop-differencing timing (see test.py).
        loop_cm = tc.For_i(0, n_loop) if n_loop > 1 else contextlib.nullcontext()
        with (
            loop_cm,
            tc.tile_pool(name="wconst", bufs=1) as wconst,
            tc.tile_pool(name="opool", bufs=4) as opool,
            tc.tile_pool(name="ppool", bufs=6, space="PSUM") as ppool,
            tc.tile_pool(name="pspool", bufs=2, space="PSUM") as pspool,
            tc.tile_pool(name="scpool", bufs=1) as scpool,
            tc.tile_pool(name="sdpool", bufs=3) as sdpool,
            tc.tile_pool(name="tpool", bufs=3) as tpool,
            tc.tile_pool(name="xpool", bufs=xbufs) as xpool,
        ):
            w_chunks = []
            for k in range(KC):
                wck = wconst.tile([P, O], BF16, tag=f"wc{k}")
                w_chunks.append(wck)

            def load_ib(i0, w):
                xts = []
                for k in range(KC):
                    xt = xpool.tile([P, IBM], BF16, tag="xt")
                    eng = nc.scalar if k % 2 else nc.sync
                    eng.dma_start(out=xt[:, 0:w],
                                  in_=xT[k * P:(k + 1) * P, i0:i0 + w])
                    xts.append(xt)
                return xts

            # phase-A inputs (small, head of both rings)
            dupx_s = scpool.tile([P, L * CW], BF16, tag="dupx")
            nc.sync.dma_start(out=dupx_s[:], in_=dupx[:])
            sdat_s = scpool.tile([P, KC * NS], BF16, tag="sdat")
            nc.scalar.dma_start(out=sdat_s[:], in_=sdat[:])
            sidx_s = scpool.tile([P, KC * NS], I16, tag="sidx")
            nc.scalar.dma_start(out=sidx_s[:], in_=sidx[:])
            abT_s = scpool.tile([64, O], BF16, tag="abT")
            nc.scalar.dma_start(out=abT_s[:], in_=abT[:])
            bm_s = scpool.tile([64, D], BF16, tag="bm")
            nc.sync.dma_start(out=bm_s[:], in_=bm[:])

            xts0 = load_ib(0, IBH)

            # dup-fold (3 adds, acc reused in place)
            acc = scpool.tile([P, CW], BF16, tag="acc")
            nc.vector.tensor_add(acc[:], dupx_s[:, 0:CW], dupx_s[:, CW:2 * CW])
            nc.vector.tensor_add(acc[:], acc[:], dupx_s[:, 2 * CW:3 * CW])
            sd3 = sdat_s[:].rearrange("p (k n) -> p k n", n=NS)[:, :, NS0:NS]
            ac3 = acc[:].rearrange("p (k c) -> p k c", c=C)
            lv3 = dupx_s[:, 3 * CW:4 * CW].rearrange("p (k c) -> p k c", c=C)
            nc.vector.tensor_add(sd3, ac3, lv3)

            def emit_chunk(k):
                sden = sdpool.tile([P, O], BF16, tag="sden")
                nc.gpsimd.local_scatter(
                    out_ap=sden[:],
                    data_ap=sdat_s[:, k * NS:(k + 1) * NS],
                    idxs_ap=sidx_s[:, k * NS:(k + 1) * NS],
                    channels=P, num_elems=O, num_idxs=NS)
                eng = nc.scalar if k % 2 else nc.sync
                eng.dma_start(out=w_chunks[k][:],
                              in_=wbT[k * P:(k + 1) * P, :])
                ps = pspool.tile([P, O], F32, tag="psA")
                nc.tensor.matmul(ps[:], lhsT=bm_s[:, k * P:(k + 1) * P],
                                 rhs=abT_s[:], start=True, stop=True)
                pst = tpool.tile([P, O], BF16, tag="pst")
                if k % 2 == 0:
                    nc.scalar.copy(out=pst[:], in_=ps[:])
                else:
                    nc.vector.tensor_copy(pst[:], ps[:])
                nc.vector.tensor_add(w_chunks[k][:], w_chunks[k][:], pst[:])
                nc.vector.tensor_add(w_chunks[k][:], w_chunks[k][:], sden[:])

            def mm_group(psl, xts, m, k, nn):
                wsl = w_chunks[k][:, m * P:(m + 1) * P]
                for n in range(nn):
                    nc.tensor.matmul(
                        psl[n][:], lhsT=wsl,
                        rhs=xts[k][:, n * 512:(n + 1) * 512],
                        start=(k == 0), stop=(k == KC - 1))

            def drain(psl, i0, m):
                for j, pt in enumerate(psl):
                    ot = opool.tile([P, 512], BF16, tag="ot")
                    nc.vector.tensor_copy(ot[:], pt[:])
                    c0 = i0 + j * 512
                    nc.scalar.dma_start(
                        out=outT[m * P:(m + 1) * P, c0:c0 + 512], in_=ot[:])

            def alloc(pool, n):
                tag = "psA" if pool is pspool else "acc"
                psl = []
                for _ in range(n):
                    pt = pool.tile([P, 512], F32, tag=tag)
                    psl.append(pt)
                return psl

            # ---- head ib (1024): m0+m1 interleaved with phase A ----
            pA = alloc(ppool, 2)
            pB = alloc(ppool, 2)
            for k in range(KC):
                emit_chunk(k)
                if k >= LEAD:
                    mm_group(pA, xts0, 0, k - LEAD, 2)
                    mm_group(pB, xts0, 1, k - LEAD, 2)
            for k in range(KC - LEAD, KC):
                mm_group(pA, xts0, 0, k, 2)
                mm_group(pB, xts0, 1, k, 2)
            drain(pA, 0, 0)
            drain(pB, 0, 1)
            pA = alloc(ppool, 2)
            pB = alloc(ppool, 2)
            for k in range(KC):
                mm_group(pA, xts0, 2, k, 2)
                mm_group(pB, xts0, 3, k, 2)
            drain(pA, 0, 2)
            drain(pB, 0, 3)

            # ---- middle ibs (2048): m0+m1 k-interleaved, then m2, m3 ----
            for ibm in range(3):
                i0 = IBH + ibm * IBM
                xts = load_ib(i0, IBM)
                pA = alloc(ppool, 4)
                pB = alloc(ppool, 2) + alloc(pspool, 2)
                for k in range(KC):
                    mm_group(pA, xts, 0, k, 4)
                    mm_group(pB, xts, 1, k, 4)
                drain(pA, i0, 0)
                drain(pB, i0, 1)
                pC = alloc(ppool, 4)
                for k in range(KC):
                    mm_group(pC, xts, 2, k, 4)
                drain(pC, i0, 2)
                pD = alloc(ppool, 2) + alloc(pspool, 2)
                for k in range(KC):
                    mm_group(pD, xts, 3, k, 4)
                drain(pD, i0, 3)

            # ---- tail ib (1024): m01 / m23 pairs, n=2 ----
            i0 = IBH + 3 * IBM
            xts = load_ib(i0, IBH)
            pA = alloc(ppool, 2)
            pB = alloc(ppool, 2)
            for k in range(KC):
                mm_group(pA, xts, 0, k, 2)
                mm_group(pB, xts, 1, k, 2)
            drain(pA, i0, 0)
            drain(pB, i0, 1)
            pA = alloc(pspool, 2)
            pB = alloc(ppool, 2)
            for k in range(KC):
                mm_group(pA, xts, 2, k, 2)
                mm_group(pB, xts, 3, k, 2)
            drain(pA, i0, 2)
            drain(pB, i0, 3)

    nc.compile()
    return nc


def _cumcount(keys):
    order = np.argsort(keys, kind="stable")
    ks = keys[order]
    _, st, ct = np.unique(ks, return_index=True, return_counts=True)
    oc = np.arange(len(ks)) - np.repeat(st, ct)
    res = np.empty(len(keys), dtype=np.int64)
    res[order] = oc
    return res


def _host_prep(x, W_inner, A, Bmat, sp_values, sp_indices):
    """Shard + layout-prep full inputs -> per-core in_maps.

    Pure layout/index manipulation; no value arithmetic beyond dtype cast.
    """
    x2 = np.asarray(x, dtype=np.float32).reshape(NI, D)
    xT = np.ascontiguousarray(x2.T).astype(ml_dtypes.bfloat16)
    W = np.asarray(W_inner, dtype=np.float32)
    A = np.asarray(A, dtype=np.float32)
    B = np.asarray(Bmat, dtype=np.float32)
    vals = np.asarray(sp_values, dtype=np.float32)
    spi = np.asarray(sp_indices)          # to host before slicing: indexing a
    rows = spi[0].astype(np.int64)        # jax array would trigger a neuron
    cols = spi[1].astype(np.int64)        # jit compile of dynamic_slice
    bmx = B.astype(ml_dtypes.bfloat16)

    in_maps = []
    for c in range(N_CORES):
        o0 = c * O
        wbT = np.ascontiguousarray(W[o0:o0 + O, :].T).astype(ml_dtypes.bfloat16)
        abT = np.ascontiguousarray(A[o0:o0 + O, :].T).astype(ml_dtypes.bfloat16)

        msk = (rows >= o0) & (rows < o0 + O)
        d = cols[msk]
        o = rows[msk] - o0
        v = vals[msk]
        slot = d * O + o
        order = np.argsort(slot, kind="stable")
        ds, os_, vs, slots = d[order], o[order], v[order], slot[order]
        uniq, starts, counts = np.unique(slots, return_index=True,
                                         return_counts=True)
        occ = np.arange(len(slots)) - np.repeat(starts, counts)
        mult = np.repeat(counts, counts)
        assert counts.max() <= L, (
            f"core {c}: COO multiplicity {counts.max()} exceeds L={L}")

        sdat = np.zeros((P, KC * NS), dtype=np.float32)
        sidx = np.full((P, KC * NS), -1, dtype=np.int16)
        dupx = np.zeros((P, L * CW), dtype=np.float32)

        dk = (ds // P).astype(np.int64)
        dp = (ds % P).astype(np.int64)
        bucket = dp * KC + dk

        uq = mult == 1
        posu = _cumcount(np.where(uq, bucket, -1))
        if uq.any():
            assert posu[uq].max() < NS0, (
                f"core {c}: {posu[uq].max() + 1} unique entries/row > {NS0}")
        pu, ku, qu = dp[uq], dk[uq], posu[uq]
        sdat[pu, ku * NS + qu] = vs[uq]
        sidx[pu, ku * NS + qu] = os_[uq] + O * (ku % SG)

        rep = (occ == 0) & (mult > 1)
        posd = _cumcount(np.where(rep, bucket, -1))
        if rep.any():
            assert posd[rep].max() < C, (
                f"core {c}: {posd[rep].max() + 1} dup slots/row > {C}")
        dslot_col = np.full(len(slots), -1, dtype=np.int64)
        dslot_col[rep] = posd[rep]
        grp = np.repeat(np.arange(len(uniq)), counts)
        rep_col = np.full(len(uniq), -1, dtype=np.int64)
        rep_col[counts > 1] = dslot_col[starts[counts > 1]]
        ecol = rep_col[grp]
        dup = mult > 1
        pd_, kd_, cd_, ld_ = dp[dup], dk[dup], ecol[dup], occ[dup]
        dupx[pd_, ld_ * CW + kd_ * C + cd_] = vs[dup]
        pr, kr, cr = dp[rep], dk[rep], posd[rep]
        sidx[pr, kr * NS + NS0 + cr] = os_[rep] + O * (kr % SG)

        in_maps.append({
            "xT": xT, "wbT": wbT, "abT": abT, "bm": bmx,
            "sdat": sdat.astype(ml_dtypes.bfloat16),
            "sidx": sidx,
            "dupx": dupx.astype(ml_dtypes.bfloat16),
        })
    return in_maps


def kernel(x, W_inner, A, Bmat, sp_values, sp_indices):
    from concourse.bass_utils import run_bass_kernel_spmd

    in_maps = _host_prep(x, W_inner, A, Bmat, sp_values, sp_indices)
    if "nc" not in _COMPILED:
        _COMPILED["nc"] = _build()
    res = run_bass_kernel_spmd(_COMPILED["nc"], in_maps,
                               core_ids=list(range(N_CORES)))
    full = np.empty((NI, D_OUT), dtype=np.float32)
    for c in range(N_CORES):
        full[:, c * O:(c + 1) * O] = res.results[c]["outT"].T.astype(np.float32)
    return full.reshape(np.asarray(x).shape[:-1] + (D_OUT,))
